# revision 29
# baseline (speedup 1.0000x reference)
"""Multi-head attention (B=2, S=2048, D=1024, H=16) on 8 Trainium2 cores.

Sharding: 2 heads per core (tensor-parallel on H). Each core computes its
2 heads' QKV projections, attention, and a partial output projection
(the 128 columns of the concat dim it owns); the host sums the 8 partial
outputs and adds the output bias.

Device dataflow per (batch, head):
  qT/kT = W x^T         [64, S]   (x^T supplied by host, bf16)
  v_nat = (xv^T)^T Wv^T [t-block 128, e 128] direct natural-layout v
                        (bias bv folded into the host-side output bias)
  sT    = kT^T q        [t-block 128, s 512] transposed scores (psum)
  expS  = exp(sT/8)     (ScalarE, direct from psum)
  o~T/Z = [v|1]^T expS  [65, s]  (P@V with ones column -> row 64 = Z)
  oT    = o~T * (1/Z)   (DVE recip + Pool partition_broadcast + DVE mul)
  y_c   = oT^T Wo_c^T   [s-block 128, 1024] partial fp16 output
                        (psum -> sbuf fp16 on Pool -> DRAM)

The emission is software-pipelined around the in-order engine queues:
scores run two t-blocks ahead of P@V (so PE never waits on the Exp),
the previous s-chunk's normalize + output projection are interleaved
into the current chunk's inner loop, and the NEXT batch's projection
chains + input DMA are spread across the current batch's four s-chunk
steps.  Projections run on bf16 operands straight from DMA (same PE
rate as f32r, no cast traffic); attention internals stay f32r.
"""

import os
import numpy as np
import ml_dtypes

B, S, D, H = 2, 2048, 1024, 16
HD = D // H          # 64
NCORES = 8
HPC = H // NCORES    # 2 heads per core
P = 128
SC = 512             # s-chunk width
NSC = S // SC        # 4
NKB = D // P         # 8 contraction blocks for projections
NTB = S // P         # 16 t-blocks

_BF16 = ml_dtypes.bfloat16

_nc_cache = {}
_runner_cache = {}


def build_nc(loop_k: int = 1, unroll: bool = False):
    """Build (and cache) the per-core Bass module. loop_k>1 wraps the body
    in a hardware loop for timing measurements (or unrolls it when
    unroll=True, used by the offline timeline simulator)."""
    cache_key = (loop_k, unroll)
    if cache_key in _nc_cache:
        return _nc_cache[cache_key]

    import concourse.bass as bass
    import concourse.mybir as mybir
    import concourse.tile as tile
    from concourse import bacc
    from contextlib import ExitStack

    f32 = mybir.dt.float32
    f32r = mybir.dt.float32r
    bf16 = mybir.dt.bfloat16
    fp16 = mybir.dt.float16
    AF = mybir.ActivationFunctionType

    nc = bacc.Bacc("TRN2", target_bir_lowering=False)

    xqT = nc.dram_tensor("xqT", [B, D, S], bf16, kind="ExternalInput")
    xkT = nc.dram_tensor("xkT", [B, D, S], bf16, kind="ExternalInput")
    xvT = nc.dram_tensor("xvT", [B, D, S], bf16, kind="ExternalInput")
    wq = nc.dram_tensor("wq", [D, P], bf16, kind="ExternalInput")
    wk = nc.dram_tensor("wk", [D, P], bf16, kind="ExternalInput")
    wv = nc.dram_tensor("wv", [D, P], bf16, kind="ExternalInput")
    bq = nc.dram_tensor("bq", [P, 1], f32, kind="ExternalInput")
    bk = nc.dram_tensor("bk", [P, 1], f32, kind="ExternalInput")
    wo = nc.dram_tensor("wo", [P, D], f32, kind="ExternalInput")
    ypart = nc.dram_tensor("ypart", [B, S, D], fp16, kind="ExternalOutput")

    with tile.TileContext(nc) as tc:
        with ExitStack() as ctx:
            const = ctx.enter_context(tc.tile_pool(name="const", bufs=1))
            xin = ctx.enter_context(tc.tile_pool(name="xin", bufs=4))
            qkv = ctx.enter_context(tc.tile_pool(name="qkv", bufs=2))
            otp = ctx.enter_context(tc.tile_pool(name="otp", bufs=3))
            vap = ctx.enter_context(tc.tile_pool(name="vap", bufs=2))
            expp = ctx.enter_context(tc.tile_pool(name="expp", bufs=4))
            smalls = ctx.enter_context(tc.tile_pool(name="smalls", bufs=4))
            yout = ctx.enter_context(tc.tile_pool(name="yout", bufs=4))
            # PSUM (8 banks): "pp" holds scores/proj/v_nat tiles
            # (slot = [128,1024] fp32 = 2 banks, 2 bufs = 4 banks);
            # "ppo" the two per-head P@V accumulators (2 banks);
            # "pyb" the output-projection tiles (2 banks).
            pp = ctx.enter_context(tc.tile_pool(name="pp", bufs=2, space="PSUM"))
            ppo = ctx.enter_context(tc.tile_pool(name="ppo", bufs=2, space="PSUM"))
            pyb = ctx.enter_context(tc.tile_pool(name="pyb", bufs=2, space="PSUM"))

            # ---- constants (outside the timing loop) ----
            wq_sb = const.tile([P, NKB, P], bf16, tag="wq")
            wk_sb = const.tile([P, NKB, P], bf16, tag="wk")
            wv_sb = const.tile([P, NKB, P], bf16, tag="wv")
            nc.sync.dma_start(wq_sb[:], wq.ap().rearrange("(a p) e -> p a e", p=P))
            nc.sync.dma_start(wk_sb[:], wk.ap().rearrange("(a p) e -> p a e", p=P))
            nc.sync.dma_start(wv_sb[:], wv.ap().rearrange("(a p) e -> p a e", p=P))
            wo_f32 = const.tile([P, D], f32, tag="wof")
            nc.sync.dma_start(wo_f32[:], wo[:, :])
            wo_sb = const.tile([P, D], bf16, tag="wo")
            nc.vector.tensor_copy(wo_sb[:], wo_f32[:])
            bq_sb = const.tile([P, 1], f32, tag="bq")
            bk_sb = const.tile([P, 1], f32, tag="bk")
            nc.sync.dma_start(bq_sb[:], bq[:, :])
            nc.sync.dma_start(bk_sb[:], bk[:, :])
            ones_f32 = const.tile([P, HD], f32, tag="onesf")
            nc.vector.memset(ones_f32[:], 1.0)

            # ---------- emission helpers ----------

            def batch_tiles(b):
                """Allocate the per-batch projection output tiles."""
                qT_sb = qkv.tile([P, S], bf16, tag="qT")
                kT_sb = qkv.tile([P, S], bf16, tag="kT")
                v_aug = vap.tile([P, HPC, NTB, HD + 1], bf16, tag="vaug")
                nc.vector.tensor_copy(
                    v_aug[:, :, :, HD], ones_f32[:, 0:HPC * NTB]
                )
                return {"qT": qT_sb, "kT": kT_sb, "va": v_aug, "xrs": {}}

            def proj_units(b, bt):
                """Emission units (closures) for batch b's projections:
                interleaved DMA batches + PE chains, in dependency order."""

                def mk_dma(xdram, key, half):
                    def emit():
                        hsl = slice(half * (S // 2), (half + 1) * (S // 2))
                        xt = xin.tile([P, NKB, S // 2], bf16, tag="xt")
                        # three parallel input rings shorten the fill:
                        # k on the SP queue, v on Act's, q on Pool's
                        eng = {"k": nc.sync, "v": nc.scalar,
                               "q": nc.gpsimd}[key]
                        eng.dma_start(
                            xt[:],
                            xdram.ap()[b, :, hsl].rearrange(
                                "(a p) s -> p a s", p=P
                            ),
                        )
                        bt["xrs"][(key, half)] = xt
                    return emit

                def mk_qk(w_sb, b_sb, dkey, key, half, sc2):
                    def emit():
                        xrs = bt["xrs"][(key, half)]
                        sc = half * 2 + sc2
                        ps = pp.tile([P, SC], f32, tag="sc")
                        for kb in range(NKB):
                            nc.tensor.matmul(
                                ps[:], w_sb[:, kb, :],
                                xrs[:, kb, sc2 * SC:(sc2 + 1) * SC],
                                start=(kb == 0), stop=(kb == NKB - 1),
                            )
                        nc.vector.tensor_scalar_add(
                            bt[dkey][:, sc * SC:(sc + 1) * SC], ps[:], b_sb[:]
                        )
                    return emit

                def mk_v(half, q4):
                    def emit():
                        xrs = bt["xrs"][("v", half)]
                        v_aug = bt["va"]
                        vps = pp.tile([P, 4, P], f32, tag="sc")
                        for tb4 in range(4):
                            toff = (q4 * 4 + tb4) * P
                            for kb in range(NKB):
                                nc.tensor.matmul(
                                    vps[:, tb4, :],
                                    xrs[:, kb, toff:toff + P],
                                    wv_sb[:, kb, :],
                                    start=(kb == 0), stop=(kb == NKB - 1),
                                )
                        tb0 = half * 8 + q4 * 4
                        nc.vector.tensor_copy(
                            v_aug[:, 0, tb0:tb0 + 4, 0:HD], vps[:, :, 0:HD]
                        )
                        nc.vector.tensor_copy(
                            v_aug[:, 1, tb0:tb0 + 4, 0:HD], vps[:, :, HD:P]
                        )
                    return emit

                units = {}
                for h in range(2):
                    units[f"kd{h}"] = mk_dma(xkT, "k", h)
                    units[f"kc{h}0"] = mk_qk(wk_sb, bk_sb, "kT", "k", h, 0)
                    units[f"kc{h}1"] = mk_qk(wk_sb, bk_sb, "kT", "k", h, 1)
                    units[f"vd{h}"] = mk_dma(xvT, "v", h)
                    units[f"vc{h}0"] = mk_v(h, 0)
                    units[f"vc{h}1"] = mk_v(h, 1)
                    units[f"qd{h}"] = mk_dma(xqT, "q", h)
                    units[f"qc{h}0"] = mk_qk(wq_sb, bq_sb, "qT", "q", h, 0)
                    units[f"qc{h}1"] = mk_qk(wq_sb, bq_sb, "qT", "q", h, 1)
                return units

            # Persistent P@V accumulators (psum) and double-buffered oT:
            # each attention step accumulates into the same two psum tiles
            # and the normalize of step i runs at the start of step i+1
            # (wrapping across For_i iterations).
            o_h0 = ppo.tile([HD + 1, SC], f32, tag="oacc")
            o_h1 = ppo.tile([HD + 1, SC], f32, tag="oacc")
            oT_a = otp.tile([P, SC], bf16, tag="oT")
            oT_b = otp.tile([P, SC], bf16, tag="oT")
            oT_tiles = [oT_a, oT_b]

            def emit_norm(prev):
                """Normalize the previous step's P@V accumulators into oT."""
                for h, o_ps in ((0, o_h0), (1, o_h1)):
                    rz = smalls.tile([1, SC], f32, tag="rz")
                    with nc.allow_low_precision(
                        reason="softmax denominator reciprocal"
                    ):
                        nc.vector.reciprocal(rz[:], o_ps[HD:HD + 1, :])
                    bcst = smalls.tile([HD, SC], f32, tag="bcs")
                    nc.gpsimd.partition_broadcast(bcst[:], rz[:])
                    nc.vector.tensor_mul(
                        prev["oT"][h * HD:(h + 1) * HD, :],
                        o_ps[0:HD, :], bcst[:],
                    )

            def emit_yout(prev, sbl):
                """Output projection for one 128-row block of the previous
                step's oT (psum -> fp16 sbuf on DVE); the whole s-chunk is
                written with a single DMA from the Pool queue so input
                prefetch on the SP queue is never blocked behind drains."""
                bp, scp = prev["b"], prev["sc"]
                if sbl == 0:
                    ysb_t = yout.tile([P, NSC, D], fp16, tag="y")
                    prev["ysb"] = ysb_t
                ysb = prev["ysb"]
                osl = prev["oT"][:, sbl * P:(sbl + 1) * P]
                for half in range(2):
                    psy = pyb.tile([P, SC], f32, tag="py")
                    nc.tensor.matmul(
                        psy[:], osl, wo_sb[:, half * SC:(half + 1) * SC],
                        start=True, stop=True,
                    )
                    nc.vector.tensor_copy(
                        ysb[:, sbl, half * SC:(half + 1) * SC], psy[:]
                    )
                if sbl == 3:
                    nc.gpsimd.dma_start(
                        ypart.ap()[bp, scp * SC:(scp + 1) * SC, :].rearrange(
                            "(a p) d -> p a d", p=P
                        ),
                        ysb[:],
                    )

            def att_step(b, sc, bt, prev, sched, step_i):
                """One s-chunk of attention with the previous step's
                normalize/output-projection and the next batch's projection
                units interleaved.  Scores run 2 t-blocks ahead of P@V."""
                qT_sb, kT_sb, v_aug = bt["qT"], bt["kT"], bt["va"]
                ssl = slice(sc * SC, (sc + 1) * SC)
                oT_sb = oT_tiles[step_i % 2]
                exs = [None] * NTB

                def s_pair(tb):
                    tsl = slice(tb * P, (tb + 1) * P)
                    ps_sc = pp.tile([P, 2 * SC], f32, tag="sc")
                    nc.tensor.matmul(
                        ps_sc[:, 0:SC], kT_sb[0:HD, tsl], qT_sb[0:HD, ssl],
                        start=True, stop=True, tile_position=(0, 0),
                    )
                    nc.tensor.matmul(
                        ps_sc[:, SC:2 * SC], kT_sb[HD:P, tsl], qT_sb[HD:P, ssl],
                        start=True, stop=True, tile_position=(64, 0),
                    )
                    ex = expp.tile([P, 2 * SC], bf16, tag="ex")
                    nc.scalar.activation(ex[:], ps_sc[:], AF.Exp, scale=0.125)
                    exs[tb] = ex

                def pv(tb):
                    nc.tensor.matmul(
                        o_h0[:], v_aug[:, 0, tb, :], exs[tb][:, 0:SC],
                        start=(tb == 0), stop=(tb == NTB - 1),
                    )
                    nc.tensor.matmul(
                        o_h1[:], v_aug[:, 1, tb, :], exs[tb][:, SC:2 * SC],
                        start=(tb == 0), stop=(tb == NTB - 1),
                    )

                def emit(pos):
                    for u in sched.get(pos, ()):
                        u()

                emit("pre")  # projection units whose inputs are resident
                s_pair(0)    # (may include the qT chunk scores read next)
                s_pair(1)
                if prev is not None:
                    emit_norm(prev)
                emit("norm")                # PE work to cover the norm drain
                for tb in range(NTB):
                    pv(tb)
                    if tb + 2 < NTB:
                        s_pair(tb + 2)
                    if prev is not None and tb in (2, 4, 6, 8):
                        emit_yout(prev, (tb - 2) // 2)
                    if tb in (8, 12):
                        emit(tb)
                return {"b": b, "sc": sc, "oT": oT_sb}

            # ---------- software pipeline (forward-only deps) ----------
            # Body: batch-0 projections, then 8 attention steps with
            # batch-1's projections interleaved into batch-0's steps;
            # the final step's normalize/output-projection drains at the
            # body tail (overlapped with the next iteration's start by
            # the dataflow scheduler where possible).
            def body():
                bts0 = batch_tiles(0)
                u0 = proj_units(0, bts0)
                # up front: only what attention step (b0,sc0) needs -- all
                # DMAs first so the input queue streams back-to-back, then
                # k/v chains and the first q chunk; remaining q chunks land
                # inside the early steps, shortening the fill
                for nm in ("kd0", "kd1", "vd0", "vd1", "qd0", "qd1",
                           "kc00", "kc01", "kc10", "kc11",
                           "vc00", "vc01", "vc10", "vc11", "qc00"):
                    u0[nm]()
                bts1 = batch_tiles(1)
                u1 = proj_units(1, bts1)
                # explicit unit schedule: every chain runs at least one
                # step after its input DMA triggers so the PE never waits
                # on an in-flight transfer
                scheds = [
                    {"norm": [u0["qc01"]],
                     8: [u0["qc10"]], 12: [u0["qc11"]]},
                    {"pre": [u1["kd0"], u1["kd1"]]},
                    {"pre": [u1["kc00"], u1["kc01"]], "norm": [u1["vd0"]],
                     8: [u1["kc10"]], 12: [u1["kc11"]]},
                    {"pre": [u1["vd1"], u1["qd0"]], "norm": [u1["vc00"]],
                     8: [u1["vc01"]], 12: [u1["vc10"]]},
                    {"pre": [u1["vc11"], u1["qc00"]], "norm": [u1["qc01"]],
                     8: [u1["qd1"]]},
                    {"pre": [u1["qc10"]], 8: [u1["qc11"]]},
                    {},
                    {},
                ]
                prev = None
                for i, (b, sc) in enumerate(
                    [(b, sc) for b in range(B) for sc in range(NSC)]
                ):
                    bt = bts1 if b == 1 else bts0
                    prev = att_step(b, sc, bt, prev, scheds[i], i)
                emit_norm(prev)
                for sbl in range(4):
                    emit_yout(prev, sbl)

            if loop_k == 1:
                body()
            elif unroll:
                for _ in range(loop_k):
                    body()
            else:
                with tc.For_i(
                    0, loop_k, 1,
                    staggered_reset=True,
                    hint_engines=(
                        mybir.EngineType.PE,
                        mybir.EngineType.DVE,
                        mybir.EngineType.Activation,
                        mybir.EngineType.SP,
                        mybir.EngineType.Pool,
                    ),
                ):
                    body()

    nc.compile()
    _nc_cache[cache_key] = nc
    return nc


def make_in_maps(inputs):
    """Host-side sharding: transpose activations to [B, D, S] bf16, slice
    per-head weights per core."""
    query, key, value = inputs["query"], inputs["key"], inputs["value"]
    Wq, bq, Wk, bk, Wv, bv = (
        inputs["Wq"], inputs["bq"], inputs["Wk"], inputs["bk"],
        inputs["Wv"], inputs["bv"],
    )
    Wo, bo = inputs["Wo"], inputs["bo"]

    xqT = np.ascontiguousarray(np.transpose(query, (0, 2, 1))).astype(_BF16)
    xkT = np.ascontiguousarray(np.transpose(key, (0, 2, 1))).astype(_BF16)
    xvT = np.ascontiguousarray(np.transpose(value, (0, 2, 1))).astype(_BF16)

    in_maps = []
    for c in range(NCORES):
        hs = slice(c * HPC, (c + 1) * HPC)
        # [HPC, HD, D] -> [D, HPC*HD]
        wq_c = np.ascontiguousarray(
            Wq[hs].reshape(HPC * HD, D).T).astype(_BF16)
        wk_c = np.ascontiguousarray(
            Wk[hs].reshape(HPC * HD, D).T).astype(_BF16)
        wv_c = np.ascontiguousarray(
            Wv[hs].reshape(HPC * HD, D).T).astype(_BF16)
        bq_c = np.ascontiguousarray(bq[hs].reshape(P, 1)).astype(np.float32)
        bk_c = np.ascontiguousarray(bk[hs].reshape(P, 1)).astype(np.float32)
        wo_c = np.ascontiguousarray(Wo[:, c * P:(c + 1) * P].T).astype(np.float32)
        in_maps.append({
            "xqT": xqT, "xkT": xkT, "xvT": xvT,
            "wq": wq_c, "wk": wk_c, "wv": wv_c,
            "bq": bq_c, "bk": bk_c,
            "wo": wo_c,
        })
    return in_maps


def make_runner(nc, n_cores=NCORES):
    """Cached jitted shard_map runner (mirrors bass2jax.run_bass_via_pjrt
    without donation so it can be re-invoked for timing)."""
    key = id(nc)
    if key in _runner_cache:
        return _runner_cache[key]
    import jax
    from jax.sharding import Mesh, PartitionSpec
    from jax.experimental.shard_map import shard_map
    import concourse.mybir as mybir
    from concourse import bass2jax

    bass2jax.install_neuronx_cc_hook()
    partition_name = nc.partition_id_tensor.name if nc.partition_id_tensor else None
    in_names, out_names, out_avals = [], [], []
    for alloc in nc.m.functions[0].allocations:
        if not isinstance(alloc, mybir.MemoryLocationSet):
            continue
        name = alloc.memorylocations[0].name
        if alloc.kind == "ExternalInput":
            if name != partition_name:
                in_names.append(name)
        elif alloc.kind == "ExternalOutput":
            out_names.append(name)
            out_avals.append(
                jax.core.ShapedArray(
                    tuple(alloc.tensor_shape), mybir.dt.np(alloc.dtype))
            )
    all_in_names = list(in_names) + ([partition_name] if partition_name else [])

    def _body(*args):
        operands = list(args)
        if partition_name is not None:
            operands.append(bass2jax.partition_id_tensor())
        outs = bass2jax._bass_exec_p.bind(
            *operands, out_avals=tuple(out_avals),
            in_names=tuple(all_in_names), out_names=tuple(out_names),
            lowering_input_output_aliases=(),
            sim_require_finite=False, sim_require_nnan=False, nc=nc)
        return tuple(outs)

    devices = jax.devices()[:n_cores]
    mesh = Mesh(np.asarray(devices), ("core",))
    fn = jax.jit(shard_map(
        _body, mesh=mesh,
        in_specs=(PartitionSpec("core"),) * len(in_names),
        out_specs=(PartitionSpec("core"),) * len(out_names),
        check_rep=False))
    out = (fn, in_names, out_names, out_avals)
    _runner_cache[key] = out
    return out


def run_on_cores(nc, in_maps):
    """Run the module on the 8 cores; returns list of per-core out dicts."""
    import jax
    fn, in_names, out_names, out_avals = make_runner(nc)
    concat_in = [
        np.concatenate([m[nm] for m in in_maps], axis=0) for nm in in_names
    ]
    outs = jax.block_until_ready(fn(*concat_in))
    res = []
    for c in range(len(in_maps)):
        d = {}
        for i, nm in enumerate(out_names):
            shp = out_avals[i].shape
            d[nm] = np.asarray(outs[i]).reshape(len(in_maps), *shp)[c]
        res.append(d)
    return res


def postprocess(results, inputs):
    """Sum per-core partial fp16 outputs and add the output bias.

    The device computes attention over un-biased v; since softmax rows sum
    to one, the v bias contributes exactly Wo @ bv_flat to every output
    row, which is folded in here."""
    acc = np.zeros((B, S, D), dtype=np.float64)
    for r in results:
        acc += r["ypart"].astype(np.float64)
    bv_flat = inputs["bv"].astype(np.float64).reshape(D)
    acc += inputs["bo"].astype(np.float64)
    acc += inputs["Wo"].astype(np.float64) @ bv_flat
    return acc.astype(np.float32)


def kernel(**inputs) -> np.ndarray:
    inputs = {k: np.asarray(v) for k, v in inputs.items()}
    nc = build_nc(loop_k=1)
    in_maps = make_in_maps(inputs)
    results = run_on_cores(nc, in_maps)
    return postprocess(results, inputs)
